# revision 17
# baseline (speedup 1.0000x reference)
"""Linear-attention kernel (out = (relu(Q)+eps) @ ((relu(K)+eps)^T V)) on 8 TRN2 cores.

Sharding: data-parallel over batch B=8 -> one batch per NeuronCore, no comm.
Per core: S=4096, D=256, DV=256, fp32 in/out, f32r (fp22) matmuls.
"""

from contextlib import ExitStack

import numpy as np

import concourse.bacc as bacc
import concourse.bass as bass
import concourse.mybir as mybir
from concourse.bass_utils import run_bass_kernel_spmd
from concourse.masks import make_identity
from concourse.tile import TileContext

B, S, D, DV = 8, 4096, 256, 256
P = 128
NCH = S // P            # 32 chunks of 128 sequence rows
GRP = 8                 # chunks per DMA group (1 MiB transfers)
NGRP = NCH // GRP       # 4
EPS = 1e-6
F32 = mybir.dt.float32
F32R = mybir.dt.float32r
RELU = mybir.ActivationFunctionType.Relu
MAX = mybir.AluOpType.max
ADD = mybir.AluOpType.add

_CACHE: dict = {}


def _build() -> bass.Bass:
    nc = bacc.Bacc("TRN2", target_bir_lowering=False)
    Kd = nc.declare_dram_parameter("K", [S, D], F32R, isOutput=False)
    Vd = nc.declare_dram_parameter("V", [S, DV], F32R, isOutput=False)
    Qd = nc.declare_dram_parameter("Q", [S, D], F32, isOutput=False)
    Od = nc.declare_dram_parameter("out", [S, DV], F32, isOutput=True)

    # seq row index s = p*NCH + n: partition-major so each partition's DMA
    # span is contiguous in DRAM (32 KB per partition for a full tensor).
    Kv = Kd[:, :].rearrange("(p n) d -> p n d", p=P)
    Vv = Vd[:, :].rearrange("(p n) d -> p n d", p=P)
    Qv = Qd[:, :].rearrange("(p n) d -> p n d", p=P)
    Ov = Od[:, :].rearrange("(p n) d -> p n d", p=P)

    with TileContext(nc) as tc, ExitStack() as ctx:
        consts = ctx.enter_context(tc.tile_pool(name="consts", bufs=1))
        big = ctx.enter_context(tc.tile_pool(name="big", bufs=1))
        pkv = ctx.enter_context(tc.tile_pool(name="pkv", bufs=1, space="PSUM"))
        pqt = ctx.enter_context(tc.tile_pool(name="pqt", bufs=2, space="PSUM"))
        pout = ctx.enter_context(tc.tile_pool(name="pout", bufs=2, space="PSUM"))

        ident = consts.tile([P, P], F32, name="ident")
        make_identity(nc, ident)
        epsb = consts.tile([P, 1], F32, name="epsb")
        nc.vector.memset(epsb, EPS)

        # Per-group tiles: one DMA writer each, keeps per-instruction sync
        # wait counts within the ISA limit.
        kts = [big.tile([P, GRP, D], F32R, name=f"kt{g}") for g in range(NGRP)]
        vts = [big.tile([P, GRP, DV], F32R, name=f"vt{g}") for g in range(NGRP)]
        qts = [big.tile([P, GRP, D], F32, name=f"qt{g}") for g in range(NGRP)]
        qtT = big.tile([P, NCH, D], F32R, name="qtT")  # (relu(Q)+eps)^T tiles
        ot = big.tile([P, NCH, DV], F32, name="ot")    # output staging
        kv = big.tile([P, 2, DV], F32R, name="kv")     # KV = K_^T V, d-halves

        # Loads: K/V groups first (phase 1 + KV gate everything), Q after.
        for g in range(NGRP):
            s = slice(g * GRP, (g + 1) * GRP)
            nc.sync.dma_start(out=kts[g][:, :, :], in_=Kv[:, s, :])
            nc.sync.dma_start(out=vts[g][:, :, :], in_=Vv[:, s, :])
        for g in range(NGRP):
            s = slice(g * GRP, (g + 1) * GRP)
            nc.sync.dma_start(out=qts[g][:, :, :], in_=Qv[:, s, :])

        # K_ = relu(K)+eps in place (DVE, fused max+add)
        for g in range(NGRP):
            nc.vector.tensor_scalar(
                out=kts[g][:, :, :], in0=kts[g][:, :, :],
                scalar1=0.0, scalar2=EPS, op0=MAX, op1=ADD,
            )

        # Phase 1: KV[d, v] = sum_k K_[k, d] * V[k, v], accumulated in PSUM.
        kvps = [pkv.tile([P, DV], F32, name=f"kvps{h}") for h in range(2)]
        for n in range(NCH):
            g, j = divmod(n, GRP)
            for h in range(2):
                nc.tensor.matmul(
                    kvps[h][:, :],
                    kts[g][:, j, h * P:(h + 1) * P],
                    vts[g][:, j, :],
                    start=(n == 0), stop=(n == NCH - 1),
                )
        for h in range(2):
            nc.vector.tensor_copy(kv[:, h, :], kvps[h][:, :])

        # Phase 2: per chunk, transpose Q tiles on PE, relu on copyback (ACT),
        # then out_chunk = Q_^T.T @ KV accumulated over the two d-halves.
        for n in range(NCH):
            g, j = divmod(n, GRP)
            for h in range(2):
                ps_t = pqt.tile([P, P], F32, name="ps_t")
                nc.tensor.transpose(ps_t[:, :], qts[g][:, j, h * P:(h + 1) * P], ident)
                nc.scalar.activation(
                    qtT[:, n, h * P:(h + 1) * P], ps_t[:, :], RELU, bias=epsb[:, :],
                )
            ps_o = pout.tile([P, DV], F32, name="ps_o")
            for h in range(2):
                nc.tensor.matmul(
                    ps_o[:, :],
                    qtT[:, n, h * P:(h + 1) * P],
                    kv[:, h, :],
                    start=(h == 0), stop=(h == 1),
                )
            nc.vector.tensor_copy(ot[:, n, :], ps_o[:, :])
            if n % GRP == GRP - 1:
                g = n // GRP
                s = slice(g * GRP, (g + 1) * GRP)
                nc.sync.dma_start(out=Ov[:, s, :], in_=ot[:, s, :])

    nc.compile()
    return nc


def _run(Q, K, V, trace=False, **trace_kwargs):
    if "nc" not in _CACHE:
        _CACHE["nc"] = _build()
    nc = _CACHE["nc"]
    Q = np.ascontiguousarray(np.asarray(Q, dtype=np.float32))
    K = np.ascontiguousarray(np.asarray(K, dtype=np.float32))
    V = np.ascontiguousarray(np.asarray(V, dtype=np.float32))
    in_maps = [{"Q": Q[b], "K": K[b], "V": V[b]} for b in range(B)]
    res = run_bass_kernel_spmd(
        nc, in_maps, core_ids=list(range(B)), trace=trace, **trace_kwargs
    )
    out = np.stack([res.results[b]["out"] for b in range(B)], axis=0)
    return out, res


def kernel(Q, K, V):
    out, _ = _run(Q, K, V, trace=False)
    return out


# revision 23
# speedup vs baseline: 1.2618x; 1.2618x over previous
"""Linear-attention kernel (out = (relu(Q)+eps) @ ((relu(K)+eps)^T V)) on 8 TRN2 cores.

Sharding: data-parallel over batch B=8 -> one batch per NeuronCore, no comm.
Per core: S=4096, D=256, DV=256, fp32 in/out.

Numerics: matmul stationary operands (relu'd K tiles, transposed relu'd Q
tiles) in fp16 (fast FWL weight loads on the PE), moving operands (V, KV) in
f32r (fp22, 1 cycle/row at N=256), fp32 PSUM accumulation.
"""

from contextlib import ExitStack

import numpy as np

import concourse.bacc as bacc
import concourse.bass as bass
import concourse.mybir as mybir
from concourse.bass_utils import run_bass_kernel_spmd
from concourse.masks import make_identity
from concourse.tile import TileContext

B, S, D, DV = 8, 4096, 256, 256
P = 128
NCH = S // P            # 32 chunks of 128 sequence rows
GRP = 4                 # chunks per DMA piece (512 KiB) / bulk-op group
NGRP = NCH // GRP       # 8
EPS = 1e-6
F32 = mybir.dt.float32
F32R = mybir.dt.float32r
F16 = mybir.dt.float16
RELU = mybir.ActivationFunctionType.Relu
MAX = mybir.AluOpType.max
ADD = mybir.AluOpType.add

_CACHE: dict = {}


def _build(wdt=F16) -> bass.Bass:
    """wdt: dtype of the stationary (weight) operands: F16 (fast) or F32R."""
    nc = bacc.Bacc("TRN2", target_bir_lowering=False)
    Kd = nc.declare_dram_parameter("K", [S, D], F32, isOutput=False)
    Vd = nc.declare_dram_parameter("V", [S, DV], F32, isOutput=False)
    Qd = nc.declare_dram_parameter("Q", [S, D], F32, isOutput=False)
    Od = nc.declare_dram_parameter("out", [S, DV], F32, isOutput=True)

    # seq row index s = p*NCH + n: partition-major so each partition's DMA
    # span is contiguous in DRAM (32 KB per partition for a full tensor).
    Kv = Kd[:, :].rearrange("(p n) d -> p n d", p=P)
    Vv = Vd[:, :].rearrange("(p n) d -> p n d", p=P)
    Qv = Qd[:, :].rearrange("(p n) d -> p n d", p=P)
    Ov = Od[:, :].rearrange("(p n) d -> p n d", p=P)

    with TileContext(nc) as tc, ExitStack() as ctx:
        consts = ctx.enter_context(tc.tile_pool(name="consts", bufs=1))
        big = ctx.enter_context(tc.tile_pool(name="big", bufs=1))
        pkv = ctx.enter_context(tc.tile_pool(name="pkv", bufs=1, space="PSUM"))
        pqt = ctx.enter_context(tc.tile_pool(name="pqt", bufs=4, space="PSUM"))
        pout = ctx.enter_context(tc.tile_pool(name="pout", bufs=2, space="PSUM"))

        ident = consts.tile([P, P], wdt, name="ident")
        make_identity(nc, ident)
        epsb = consts.tile([P, 1], F32, name="epsb")
        nc.vector.memset(epsb, EPS)

        # Raw per-group staging (one DMA writer per tile).
        kraws = [big.tile([P, GRP, D], F32, name=f"kraw{g}") for g in range(NGRP)]
        qraws = [big.tile([P, GRP, D], F32, name=f"qraw{g}") for g in range(NGRP)]
        vts = [big.tile([P, GRP, DV], wdt, name=f"vt{g}") for g in range(NGRP)]
        kh = big.tile([P, NCH, D], wdt, name="kh")     # relu(K)+eps
        qh = big.tile([P, NCH, D], wdt, name="qh")     # relu(Q)+eps
        qtT = big.tile([P, NCH, D], wdt, name="qtT")   # (relu(Q)+eps)^T tiles
        ot = big.tile([P, NCH, DV], F32, name="ot")    # output staging
        kv = big.tile([P, 2, DV], wdt, name="kv")      # KV = K_^T V, d-halves

        # Loads: interleave K/V/Q in 512 KiB pieces so arrivals track issue
        # order and compute can pipeline behind the DMA stream.
        for g in range(NGRP):
            s = slice(g * GRP, (g + 1) * GRP)
            nc.sync.dma_start(out=kraws[g][:, :, :], in_=Kv[:, s, :])
            # SWDGE cast-DMA: V lands in SBUF already in the matmul dtype.
            nc.gpsimd.dma_start(out=vts[g][:, :, :], in_=Vv[:, s, :])
            nc.sync.dma_start(out=qraws[g][:, :, :], in_=Qv[:, s, :])

        kvps = [pkv.tile([P, DV], F32, name=f"kvps{h}") for h in range(2)]

        for g in range(NGRP):
            s = slice(g * GRP, (g + 1) * GRP)
            # K_ = relu(K)+eps, cast to weight dtype (DVE)
            nc.vector.tensor_scalar(
                out=kh[:, s, :], in0=kraws[g][:, :, :],
                scalar1=0.0, scalar2=EPS, op0=MAX, op1=ADD,
            )
            # Q_ = relu(Q)+eps, cast to weight dtype (ACT)
            nc.scalar.activation(qh[:, s, :], qraws[g][:, :, :], RELU, bias=epsb[:, :])
            # Phase 1: KV[d, v] += K_[k, d] * V[k, v] over this group's chunks.
            for j in range(GRP):
                n = g * GRP + j
                for h in range(2):
                    nc.tensor.matmul(
                        kvps[h][:, :],
                        kh[:, n, h * P:(h + 1) * P],
                        vts[g][:, j, :],
                        start=(n == 0), stop=(n == NCH - 1),
                    )
            # Transpose Q_ tiles on the PE while loads continue.
            for j in range(GRP):
                n = g * GRP + j
                for h in range(2):
                    ps_t = pqt.tile([P, P], wdt, name="ps_t")
                    nc.tensor.transpose(ps_t[:, :], qh[:, n, h * P:(h + 1) * P], ident)
                    nc.vector.tensor_copy(qtT[:, n, h * P:(h + 1) * P], ps_t[:, :])

        for h in range(2):
            nc.scalar.copy(kv[:, h, :], kvps[h][:, :])

        # Phase 2: out_chunk = Q_^T.T @ KV accumulated over the two d-halves.
        for n in range(NCH):
            ps_o = pout.tile([P, DV], F32, name="ps_o")
            for h in range(2):
                nc.tensor.matmul(
                    ps_o[:, :],
                    qtT[:, n, h * P:(h + 1) * P],
                    kv[:, h, :],
                    start=(h == 0), stop=(h == 1),
                )
            # Alternate copyback engine so neither ACT nor DVE bottlenecks.
            if n % 2 == 0:
                nc.vector.tensor_copy(ot[:, n, :], ps_o[:, :])
            else:
                nc.scalar.copy(ot[:, n, :], ps_o[:, :])
            if n % GRP == GRP - 1:
                g = n // GRP
                s = slice(g * GRP, (g + 1) * GRP)
                nc.sync.dma_start(out=Ov[:, s, :], in_=ot[:, s, :])

    nc.compile()
    return nc


def _run(Q, K, V, trace=False, wdt=F16, **trace_kwargs):
    key = ("nc", str(wdt))
    if key not in _CACHE:
        _CACHE[key] = _build(wdt)
    nc = _CACHE[key]
    Q = np.ascontiguousarray(np.asarray(Q, dtype=np.float32))
    K = np.ascontiguousarray(np.asarray(K, dtype=np.float32))
    V = np.ascontiguousarray(np.asarray(V, dtype=np.float32))
    in_maps = [{"Q": Q[b], "K": K[b], "V": V[b]} for b in range(B)]
    res = run_bass_kernel_spmd(
        nc, in_maps, core_ids=list(range(B)), trace=trace, **trace_kwargs
    )
    out = np.stack([res.results[b]["out"] for b in range(B)], axis=0)
    return out, res


def kernel(Q, K, V):
    out, _ = _run(Q, K, V, trace=False)
    return out
